# revision 29
# baseline (speedup 1.0000x reference)
"""LocalWindowAttention Trainium2 kernel.

Problem: B=8, S=4096, D=1024, H=16 heads, hd=64, window W=64.
  qkv = x @ qkv_w + qkv_b; per-window attention with relative position
  bias; out = attn_out @ proj_w + proj_b.

Sharding: data-parallel over batch — one batch element per NeuronCore
(8 cores), no collectives needed.

Per-core pipeline (S=4096 rows, processed in s-tiles of 512 rows):
  1. x is pre-transposed HOST-side to xT [D, S]; feature-major xt chunks
     (one tile per 128-feature block -> per-chunk dependencies) stream in
     with plain wide DMAs — no on-chip transposes.
  2. qT/kT (feature-major, one tile per m-block) and v (seq-major) via
     fp16 matmuls vs resident qkv_w tiles; fp32 PSUM accumulation.
  3. Attention per 128-row block (= 2 windows of 64) and per group of 4
     same-row-parity heads (0,2,4,6 / 1,3,5,7 / 8,10,.. / 9,11,..):
       scoresT[k,q] = kT.T @ qT   (4 heads -> col-quarters of ONE [128,512]
                                   PSUM tile; uniform stationary base
                                   partition per group — mixing base 0 and
                                   base 64 groups in one bank CRASHES hw)
       att = exp(scoresT) * expb  (expb = exp(rel_bias) fp16 table; cross-
                                   window entries exactly 0 — replaces the
                                   -1e4 additive mask; exp+mul in halves so
                                   downstream unblocks sooner)
       outT_unnorm[q,hd], denom[q] = att.T @ [v | 1]  (4 heads -> 65-col
                                   slots of ONE PSUM tile)
       attn_out[q,hd] = outT_unnorm * (1/denom)  (one strided reciprocal +
                                   one stride-0-broadcast multiply per group)
  4. attn_out PE-transposed per 128-col block; proj matmul; DMA out.

Software-pipeline inversion: s-tile N's qkv is emitted BEFORE s-tile
N-1's attention, so qt/kt drains are long done when attention needs them
and the PE never stalls at s-tile boundaries.

Scale 1/sqrt(hd) is folded into qkv_w's q-columns host-side. qkv_b's
v-part is folded into an effective proj bias host-side (rows of attn sum
to 1). All matmul operands are fp16 (error ~1e-3 vs fp32 reference);
accumulation is always fp32.

Measured dead ends (do not revisit): fp8 in any matmul breaks the 2e-2
accuracy gate (v-only fp8 ~4e-2, qk fp8 ~14e-2); DMA-transpose for
attn_out serializes the DMA stream on xbar-mode switches (+450us);
feature-major av needs a [1,512] single-partition reciprocal (3.3us on
DVE) or a two-PSUM-operand multiply (illegal, NCC_IBVF027).
"""
import numpy as np

import concourse.bacc as bacc
import concourse.mybir as mybir
from concourse.tile import TileContext
from concourse.bass_utils import run_bass_kernel_spmd
from concourse.masks import make_identity

F16 = mybir.dt.float16
F32 = mybir.dt.float32

B, S, D = 8, 4096, 1024
H, W, HD = 16, 64, 64
NW = S // W              # 64 windows
STILE = 512              # seq rows per pipeline tile
NST = S // STILE         # 8 s-tiles
NBLK = STILE // 128      # 4 row-blocks (window pairs) per s-tile

# head groups of 4 with uniform kt/qt row parity (see module docstring)
HGS = [(0, 8, 2), (1, 8, 2), (8, 16, 2), (9, 16, 2)]


def _build(n_stiles=NST, with_qkbias=False, with_projbias=False):
    nc = bacc.Bacc()
    s_total = n_stiles * STILE

    xt_ext = nc.declare_dram_parameter("xt16", [D, s_total], F16, isOutput=False)
    w_ext = nc.declare_dram_parameter("qkvw16", [D, 3 * D], F16, isOutput=False)
    pw_ext = nc.declare_dram_parameter("projw16", [D, D], F16, isOutput=False)
    eb_ext = nc.declare_dram_parameter("expb16", [128, H * 128], F16,
                                       isOutput=False)
    out_ext = nc.declare_dram_parameter("out", [s_total, D], F32, isOutput=True)
    if with_qkbias:
        qkb_ext = nc.declare_dram_parameter("qkb", [16, 128, 1], F32,
                                            isOutput=False)
    if with_projbias:
        pbb_ext = nc.declare_dram_parameter("projb_bcast", [128, D], F32,
                                            isOutput=False)

    with TileContext(nc) as tc:
        with (
            tc.tile_pool(name="const", bufs=1) as const,
            tc.tile_pool(name="xtp", bufs=2) as xtp,
            tc.tile_pool(name="qktp", bufs=2) as qktp,
            tc.tile_pool(name="vap", bufs=8) as vap,
            tc.tile_pool(name="arp", bufs=6) as arp,
            tc.tile_pool(name="attp", bufs=6) as attp,
            tc.tile_pool(name="rcp", bufs=6) as rcp,
            tc.tile_pool(name="aout", bufs=2) as aout,
            tc.tile_pool(name="atp", bufs=3) as atp,
            tc.tile_pool(name="outp", bufs=2) as outp,
            tc.tile_pool(name="acc", bufs=2, space="PSUM") as acc,
            tc.tile_pool(name="scps", bufs=3, space="PSUM") as scps,
            tc.tile_pool(name="aops", bufs=2, space="PSUM") as aops,
            tc.tile_pool(name="tp", bufs=1, space="PSUM") as tp,
        ):
            def _load_xt(s0):
                xts = []
                for c in range(8):
                    xc = xtp.tile([128, STILE], F16, name="xc", tag=f"xc{c}")
                    nc.sync.dma_start(
                        out=xc[:],
                        in_=xt_ext[c * 128:(c + 1) * 128, s0:s0 + STILE])
                    xts.append(xc)
                return xts

            # stile 0's input chunks go to the DMA queues first: the first
            # qkv matmul needs xt chunk 0 + weight block 0, nothing else
            xts0 = _load_xt(0)

            # ---- resident constants -------------------------------------
            wts = []
            for k in range(8):
                wk = const.tile([128, 3 * D], F16, name=f"wk{k}")
                nc.sync.dma_start(out=wk[:], in_=w_ext[k * 128:(k + 1) * 128, :])
                wts.append(wk)
            ident = const.tile([128, 128], F16, name="ident")
            make_identity(nc, ident)
            if with_qkbias:
                qkb = const.tile([128, 16], F32, name="qkb")
                for m in range(16):
                    nc.sync.dma_start(out=qkb[:, m:m + 1], in_=qkb_ext[m])

            # proj weights / bias-exp table aren't needed until the first
            # attention block (~40us in); late emission keeps the critical
            # xt/wts dispatches at the head of the DMA queues
            pwts, late = [], {}

            def _late_consts():
                for k in range(8):
                    pk = const.tile([128, D], F16, name=f"pk{k}")
                    nc.sync.dma_start(out=pk[:],
                                      in_=pw_ext[k * 128:(k + 1) * 128, :])
                    pwts.append(pk)
                ebt = const.tile([128, H * 128], F16, name="ebt")
                nc.sync.dma_start(out=ebt[:], in_=eb_ext[:])
                late["ebt3"] = ebt.rearrange("p (h q) -> p h q", q=128)
                if with_projbias:
                    pbb = const.tile([128, D], F32, name="pbb")
                    nc.sync.dma_start(out=pbb[:], in_=pbb_ext[:])
                    late["pbb"] = pbb

            def _attention(vts, qts, kts, s0):
                ebt3 = late["ebt3"]
                for p in range(NBLK):
                    ao4 = [aout.tile([128, 256], F16, name="ao", tag=f"ao{g}")
                           for g in range(4)]
                    vt3 = vts[p].rearrange("p (h c) -> p h c", c=HD + 1)
                    for hg in range(4):
                        heads = list(range(*HGS[hg]))
                        r = (heads[0] % 2) * 64
                        # 4 score matmuls -> col-quarters of one PSUM tile
                        # (uniform stationary base partition); readers come
                        # after all four writers
                        scb = scps.tile([128, 512], F32, name="scb")
                        for i, h in enumerate(heads):
                            c0 = p * 128
                            nc.tensor.matmul(
                                scb[:, i * 128:(i + 1) * 128],
                                kts[h // 2][r:r + 64, c0:c0 + 128],
                                qts[h // 2][r:r + 64, c0:c0 + 128],
                                start=True, stop=True)
                        # exp then *exp(bias) in halves (2 heads each) so
                        # the first av matmuls unblock sooner
                        att_h = []
                        for half in range(2):
                            hs = slice(half * 256, (half + 1) * 256)
                            araw = arp.tile([128, 256], F16, name="araw",
                                            tag=f"ar{half}")
                            nc.scalar.activation(
                                araw[:], scb[:, hs],
                                mybir.ActivationFunctionType.Exp)
                            att = attp.tile([128, 256], F16, name="att",
                                            tag=f"at{half}")
                            nc.vector.tensor_mul(
                                att.rearrange("p (h q) -> p h q", q=128),
                                araw.rearrange("p (h q) -> p h q", q=128),
                                ebt3[:, HGS[hg][0] + half * 4:
                                     min(HGS[hg][0] + (half + 1) * 4, 16):2,
                                     :])
                            att_h.append(att)
                        # 4 attention-weighted-V matmuls -> 65-col slots of
                        # one PSUM tile (col 64 of each = denominator)
                        aop4 = aops.tile([128, 4 * (HD + 1)], F32,
                                         name="aop4")
                        a3 = aop4.rearrange("p (i c) -> p i c", c=HD + 1)
                        for i, h in enumerate(heads):
                            nc.tensor.matmul(
                                aop4[:, i * (HD + 1):(i + 1) * (HD + 1)],
                                att_h[i // 2][:, (i % 2) * 128:
                                              (i % 2) * 128 + 128],
                                vt3[:, h, :],
                                start=True, stop=True)
                        rc4 = rcp.tile([128, 4], F32, name="rc4")
                        nc.vector.reciprocal(rc4[:], a3[:, :, HD])
                        ao3 = ao4[hg].rearrange("p (i c) -> p i c", c=HD)
                        nc.vector.tensor_mul(
                            ao3, a3[:, :, 0:HD],
                            rc4.rearrange("p (i j) -> p i j", j=1).broadcast_to(
                                [128, 4, HD]))

                    # stage 4: transpose attn_out, proj matmul, store.
                    # at block c = transpose of ao4[c//2]'s half (c%2) —
                    # features arrive in parity-group order (heads 0,2 /
                    # 4,6 / 1,3 / ...); proj_w's ROWS are permuted to match
                    # host-side, so the proj matmul needs no reshuffling
                    at = atp.tile([128, D], F16, name="at")
                    for c in range(8):
                        tpp = tp.tile([128, 128], F16, name="tpp")
                        nc.tensor.transpose(
                            tpp[:], ao4[c // 2][:, (c % 2) * 128:
                                                (c % 2) * 128 + 128],
                            ident[:])
                        nc.vector.tensor_copy(at[:, c * 128:(c + 1) * 128],
                                              tpp[:])
                    ot = outp.tile([128, D], F32, name="ot")
                    for n in range(2):
                        ac = acc.tile([128, STILE], F32, name="ac")
                        for k in range(8):
                            nc.tensor.matmul(
                                ac[:],
                                at[:, k * 128:(k + 1) * 128],
                                pwts[k][:, n * 512:(n + 1) * 512],
                                start=(k == 0), stop=(k == 7))
                        nc.scalar.copy(ot[:, n * 512:(n + 1) * 512], ac[:])
                    if with_projbias:
                        nc.vector.tensor_add(ot[:], ot[:], late["pbb"])
                    nc.sync.dma_start(
                        out=out_ext[s0 + p * 128:s0 + (p + 1) * 128, :],
                        in_=ot[:])

            # ---- main loop over s-tiles (+1 epilogue pass) --------------
            # s-tile N's qkv is emitted before s-tile N-1's attention
            prev_stage = None
            for st in range(n_stiles + 1):
                s0 = st * STILE
                if st == n_stiles:
                    _attention(*prev_stage)
                    break
                xts = xts0 if st == 0 else _load_xt(s0)

                # stage 2a: qT, kT (feature-major, per-m-block tiles; q,k
                # interleaved so attention head-groups unblock early)
                qts, kts = [], []
                for m in range(8):
                    for which, dsts in ((0, qts), (1, kts)):
                        ac = acc.tile([128, STILE], F32, name="ac")
                        col0 = which * D + m * 128
                        for k in range(8):
                            nc.tensor.matmul(
                                ac[:],
                                wts[k][:, col0:col0 + 128],
                                xts[k][:],
                                start=(k == 0), stop=(k == 7))
                        dt = qktp.tile([128, STILE], F16, name="qk",
                                       tag=f"qk{which}{m}")
                        if with_qkbias:
                            nc.scalar.activation(
                                dt[:], ac[:],
                                mybir.ActivationFunctionType.Identity,
                                bias=qkb[:, which * 8 + m:which * 8 + m + 1])
                        else:
                            nc.scalar.copy(dt[:], ac[:])
                        dsts.append(dt)

                if st == 0:
                    _late_consts()

                # stage 2b: v (seq-major, ones column appended per head)
                vts = []
                for b in range(NBLK):
                    vt = vap.tile([128, H * (HD + 1)], F16, name="vt")
                    vt3 = vt.rearrange("p (h c) -> p h c", c=HD + 1)
                    nc.vector.memset(vt3[:, :, HD:HD + 1], 1.0)
                    for n in range(2):
                        ac = acc.tile([128, STILE], F32, name="ac")
                        for k in range(8):
                            nc.tensor.matmul(
                                ac[:],
                                xts[k][:, b * 128:b * 128 + 128],
                                wts[k][:, 2 * D + n * 512:2 * D + (n + 1) * 512],
                                start=(k == 0), stop=(k == 7))
                        nc.vector.tensor_copy(
                            vt3[:, n * 8:(n + 1) * 8, 0:HD],
                            ac.rearrange("p (h c) -> p h c", c=HD))
                    vts.append(vt)

                cur = (vts, qts, kts, s0)
                if prev_stage is not None:
                    _attention(*prev_stage)
                prev_stage = cur

    nc.compile()
    return nc


def _host_prep(x, qkv_w, qkv_b, proj_w, proj_b, rel_bias):
    """Fold scale/biases, cast to fp16, build the exp'd blocked bias table."""
    scale = 1.0 / np.sqrt(HD)
    qkv_w_s = np.asarray(qkv_w, dtype=np.float64).copy()
    qkv_w_s[:, :D] *= scale
    qkv_b = np.asarray(qkv_b, dtype=np.float64)
    qkv_b_s = qkv_b.copy()
    qkv_b_s[:D] *= scale

    # rel-bias expanded to [H, W, W], packed into the transposed,
    # window-pair [128 (k), H*128 (h-major, q)] table, then EXP'd:
    # att = exp(scores) * exp(bias); masked cross-window entries become
    # exactly 0.
    rb = np.asarray(rel_bias, dtype=np.float32)
    coords = np.arange(W)
    rel = coords[:, None] - coords[None, :] + (W - 1)      # [q, k]
    bias_hqk = rb[rel].transpose(2, 0, 1)                  # [H, q, k]
    b2 = np.full((H, 128, 128), -10000.0, dtype=np.float32)  # [H, k2, q2]
    bias_kq = bias_hqk.transpose(0, 2, 1)                  # [H, k, q]
    b2[:, :64, :64] = bias_kq
    b2[:, 64:, 64:] = bias_kq
    expb16 = np.ascontiguousarray(
        np.exp(b2.transpose(1, 0, 2)).reshape(128, H * 128)).astype(np.float16)

    # v-bias commutes through attention (rows sum to 1) -> fold into proj_b
    proj_b_eff = (qkv_b[2 * D:] @ np.asarray(proj_w, dtype=np.float64)
                  + np.asarray(proj_b, dtype=np.float64))

    # attn_out features arrive in parity-group order (see _attention):
    # block c2 holds the head pair PAIRS[c2]; permute proj_w rows to match
    pairs = [(0, 2), (4, 6), (1, 3), (5, 7),
             (8, 10), (12, 14), (9, 11), (13, 15)]
    perm = np.array([h * HD + d for pr in pairs for h in pr
                     for d in range(HD)])
    shared = {
        "qkvw16": qkv_w_s.astype(np.float16),
        "projw16": np.ascontiguousarray(
            np.asarray(proj_w)[perm]).astype(np.float16),
        "expb16": expb16,
    }
    qk_bias = qkv_b_s[:2 * D]
    with_qkbias = bool(np.any(qk_bias))
    if with_qkbias:
        shared["qkb"] = np.ascontiguousarray(
            qk_bias.reshape(16, 128, 1).astype(np.float32))
    with_projbias = bool(np.any(proj_b_eff))
    if with_projbias:
        shared["projb_bcast"] = np.broadcast_to(
            proj_b_eff.astype(np.float32), (128, D)).copy()
    return shared, with_qkbias, with_projbias


_NC_CACHE = {}


def kernel(x, qkv_w, qkv_b, proj_w, proj_b, rel_bias):
    x = np.asarray(x)
    shared, wqk, wpb = _host_prep(x, qkv_w, qkv_b, proj_w, proj_b, rel_bias)

    key = (wqk, wpb)
    if key not in _NC_CACHE:
        _NC_CACHE[key] = _build(NST, wqk, wpb)
    nc = _NC_CACHE[key]

    # feature-major xT per batch element (seq stays the fast axis on chip)
    xt16 = np.ascontiguousarray(
        x.astype(np.float16).transpose(0, 2, 1))          # [B, D, S]
    in_maps = [dict(shared, xt16=xt16[i]) for i in range(B)]
    res = run_bass_kernel_spmd(nc, in_maps, list(range(B)))
    return np.stack([res.results[i]["out"] for i in range(B)], axis=0)


if __name__ == "__main__":
    rng = np.random.default_rng(0)
    x = rng.standard_normal((B, S, D), dtype=np.float32)
    qkv_w = rng.standard_normal((D, 3 * D), dtype=np.float32) / np.sqrt(D)
    proj_w = rng.standard_normal((D, D), dtype=np.float32) / np.sqrt(D)
    out = kernel(x, qkv_w, np.zeros(3 * D, np.float32), proj_w,
                 np.zeros(D, np.float32),
                 rng.standard_normal((2 * W - 1, H), dtype=np.float32) * 0.02)
    print(out.shape, out.dtype)


# revision 30
# speedup vs baseline: 1.0121x; 1.0121x over previous
"""LocalWindowAttention Trainium2 kernel.

Problem: B=8, S=4096, D=1024, H=16 heads, hd=64, window W=64.
  qkv = x @ qkv_w + qkv_b; per-window attention with relative position
  bias; out = attn_out @ proj_w + proj_b.

Sharding: data-parallel over batch — one batch element per NeuronCore
(8 cores), no collectives needed.

Per-core pipeline (S=4096 rows, processed in s-tiles of 512 rows):
  1. x is pre-transposed HOST-side to xT [D, S]; feature-major xt chunks
     (one tile per 128-feature block -> per-chunk dependencies) stream in
     with plain wide DMAs — no on-chip transposes.
  2. qT/kT (feature-major, one tile per m-block) and v (seq-major) via
     fp16 matmuls vs resident qkv_w tiles; fp32 PSUM accumulation.
  3. Attention per 128-row block (= 2 windows of 64) and per group of 4
     same-row-parity heads (0,2,4,6 / 1,3,5,7 / 8,10,.. / 9,11,..):
       scoresT[k,q] = kT.T @ qT   (4 heads -> col-quarters of ONE [128,512]
                                   PSUM tile; uniform stationary base
                                   partition per group — mixing base 0 and
                                   base 64 groups in one bank CRASHES hw)
       att = exp(scoresT) * expb  (expb = exp(rel_bias) fp16 table; cross-
                                   window entries exactly 0 — replaces the
                                   -1e4 additive mask; exp+mul in halves so
                                   downstream unblocks sooner)
       outT_unnorm[q,hd], denom[q] = att.T @ [v | 1]  (4 heads -> 65-col
                                   slots of ONE PSUM tile)
       attn_out[q,hd] = outT_unnorm * (1/denom)  (one strided reciprocal +
                                   one stride-0-broadcast multiply per group)
  4. attn_out PE-transposed per 128-col block; proj matmul; DMA out.

Software-pipeline inversion: s-tile N's qkv is emitted BEFORE s-tile
N-1's attention, so qt/kt drains are long done when attention needs them
and the PE never stalls at s-tile boundaries.

Scale 1/sqrt(hd) is folded into qkv_w's q-columns host-side. qkv_b's
v-part is folded into an effective proj bias host-side (rows of attn sum
to 1). All matmul operands are fp16 (error ~1e-3 vs fp32 reference);
accumulation is always fp32.

Measured dead ends (do not revisit): fp8 in any matmul breaks the 2e-2
accuracy gate (v-only fp8 ~4e-2, qk fp8 ~14e-2); DMA-transpose for
attn_out serializes the DMA stream on xbar-mode switches (+450us);
feature-major av needs a [1,512] single-partition reciprocal (3.3us on
DVE) or a two-PSUM-operand multiply (illegal, NCC_IBVF027).
"""
import numpy as np

import concourse.bacc as bacc
import concourse.mybir as mybir
from concourse.tile import TileContext
from concourse.bass_utils import run_bass_kernel_spmd
from concourse.masks import make_identity

F16 = mybir.dt.float16
F32 = mybir.dt.float32

B, S, D = 8, 4096, 1024
H, W, HD = 16, 64, 64
NW = S // W              # 64 windows
STILE = 512              # seq rows per pipeline tile
NST = S // STILE         # 8 s-tiles
NBLK = STILE // 128      # 4 row-blocks (window pairs) per s-tile

# head groups of 4 with uniform kt/qt row parity (see module docstring)
HGS = [(0, 8, 2), (1, 8, 2), (8, 16, 2), (9, 16, 2)]


def _build(n_stiles=NST, with_qkbias=False, with_projbias=False):
    nc = bacc.Bacc()
    s_total = n_stiles * STILE

    xt_ext = nc.declare_dram_parameter("xt16", [D, s_total], F16, isOutput=False)
    w_ext = nc.declare_dram_parameter("qkvw16", [D, 3 * D], F16, isOutput=False)
    pw_ext = nc.declare_dram_parameter("projw16", [D, D], F16, isOutput=False)
    eb_ext = nc.declare_dram_parameter("expb16", [128, H * 128], F16,
                                       isOutput=False)
    out_ext = nc.declare_dram_parameter("out", [s_total, D], F32, isOutput=True)
    if with_qkbias:
        qkb_ext = nc.declare_dram_parameter("qkb", [16, 128, 1], F32,
                                            isOutput=False)
    if with_projbias:
        pbb_ext = nc.declare_dram_parameter("projb_bcast", [128, D], F32,
                                            isOutput=False)

    with TileContext(nc) as tc:
        with (
            tc.tile_pool(name="const", bufs=1) as const,
            tc.tile_pool(name="xtp", bufs=2) as xtp,
            tc.tile_pool(name="qktp", bufs=2) as qktp,
            tc.tile_pool(name="vap", bufs=8) as vap,
            tc.tile_pool(name="arp", bufs=8) as arp,
            tc.tile_pool(name="attp", bufs=8) as attp,
            tc.tile_pool(name="rcp", bufs=8) as rcp,
            tc.tile_pool(name="aout", bufs=3) as aout,
            tc.tile_pool(name="atp", bufs=4) as atp,
            tc.tile_pool(name="outp", bufs=3) as outp,
            tc.tile_pool(name="acc", bufs=2, space="PSUM") as acc,
            tc.tile_pool(name="scps", bufs=3, space="PSUM") as scps,
            tc.tile_pool(name="aops", bufs=2, space="PSUM") as aops,
            tc.tile_pool(name="tp", bufs=1, space="PSUM") as tp,
        ):
            def _load_xt(s0):
                xts = []
                for c in range(8):
                    xc = xtp.tile([128, STILE], F16, name="xc", tag=f"xc{c}")
                    nc.sync.dma_start(
                        out=xc[:],
                        in_=xt_ext[c * 128:(c + 1) * 128, s0:s0 + STILE])
                    xts.append(xc)
                return xts

            # stile 0's input chunks go to the DMA queues first: the first
            # qkv matmul needs xt chunk 0 + weight block 0, nothing else
            xts0 = _load_xt(0)

            # ---- resident constants -------------------------------------
            wts = []
            for k in range(8):
                wk = const.tile([128, 3 * D], F16, name=f"wk{k}")
                nc.sync.dma_start(out=wk[:, :2 * D],
                                  in_=w_ext[k * 128:(k + 1) * 128, :2 * D])
                wts.append(wk)
            for k in range(8):
                nc.sync.dma_start(out=wts[k][:, 2 * D:],
                                  in_=w_ext[k * 128:(k + 1) * 128, 2 * D:])
            ident = const.tile([128, 128], F16, name="ident")
            make_identity(nc, ident)
            if with_qkbias:
                qkb = const.tile([128, 16], F32, name="qkb")
                for m in range(16):
                    nc.sync.dma_start(out=qkb[:, m:m + 1], in_=qkb_ext[m])

            # proj weights / bias-exp table aren't needed until the first
            # attention block (~40us in); late emission keeps the critical
            # xt/wts dispatches at the head of the DMA queues
            pwts, late = [], {}

            def _late_consts():
                for k in range(8):
                    pk = const.tile([128, D], F16, name=f"pk{k}")
                    nc.sync.dma_start(out=pk[:],
                                      in_=pw_ext[k * 128:(k + 1) * 128, :])
                    pwts.append(pk)
                ebt = const.tile([128, H * 128], F16, name="ebt")
                nc.sync.dma_start(out=ebt[:], in_=eb_ext[:])
                late["ebt3"] = ebt.rearrange("p (h q) -> p h q", q=128)
                if with_projbias:
                    pbb = const.tile([128, D], F32, name="pbb")
                    nc.sync.dma_start(out=pbb[:], in_=pbb_ext[:])
                    late["pbb"] = pbb

            def _attention(vts, qts, kts, s0):
                ebt3 = late["ebt3"]
                for p in range(NBLK):
                    ao4 = [aout.tile([128, 256], F16, name="ao", tag=f"ao{g}")
                           for g in range(4)]
                    vt3 = vts[p].rearrange("p (h c) -> p h c", c=HD + 1)
                    for hg in range(4):
                        heads = list(range(*HGS[hg]))
                        r = (heads[0] % 2) * 64
                        # 4 score matmuls -> col-quarters of one PSUM tile
                        # (uniform stationary base partition); readers come
                        # after all four writers
                        scb = scps.tile([128, 512], F32, name="scb")
                        for i, h in enumerate(heads):
                            c0 = p * 128
                            nc.tensor.matmul(
                                scb[:, i * 128:(i + 1) * 128],
                                kts[h // 2][r:r + 64, c0:c0 + 128],
                                qts[h // 2][r:r + 64, c0:c0 + 128],
                                start=True, stop=True)
                        # exp then *exp(bias) in halves (2 heads each) so
                        # the first av matmuls unblock sooner
                        att_h = []
                        for half in range(2):
                            hs = slice(half * 256, (half + 1) * 256)
                            araw = arp.tile([128, 256], F16, name="araw",
                                            tag=f"ar{half}")
                            nc.scalar.activation(
                                araw[:], scb[:, hs],
                                mybir.ActivationFunctionType.Exp)
                            att = attp.tile([128, 256], F16, name="att",
                                            tag=f"at{half}")
                            nc.vector.tensor_mul(
                                att.rearrange("p (h q) -> p h q", q=128),
                                araw.rearrange("p (h q) -> p h q", q=128),
                                ebt3[:, HGS[hg][0] + half * 4:
                                     min(HGS[hg][0] + (half + 1) * 4, 16):2,
                                     :])
                            att_h.append(att)
                        # 4 attention-weighted-V matmuls -> 65-col slots of
                        # one PSUM tile (col 64 of each = denominator)
                        aop4 = aops.tile([128, 4 * (HD + 1)], F32,
                                         name="aop4")
                        a3 = aop4.rearrange("p (i c) -> p i c", c=HD + 1)
                        for i, h in enumerate(heads):
                            nc.tensor.matmul(
                                aop4[:, i * (HD + 1):(i + 1) * (HD + 1)],
                                att_h[i // 2][:, (i % 2) * 128:
                                              (i % 2) * 128 + 128],
                                vt3[:, h, :],
                                start=True, stop=True)
                        rc4 = rcp.tile([128, 4], F32, name="rc4")
                        nc.vector.reciprocal(rc4[:], a3[:, :, HD])
                        ao3 = ao4[hg].rearrange("p (i c) -> p i c", c=HD)
                        nc.vector.tensor_mul(
                            ao3, a3[:, :, 0:HD],
                            rc4.rearrange("p (i j) -> p i j", j=1).broadcast_to(
                                [128, 4, HD]))

                    # stage 4: transpose attn_out, proj matmul, store.
                    # at block c = transpose of ao4[c//2]'s half (c%2) —
                    # features arrive in parity-group order (heads 0,2 /
                    # 4,6 / 1,3 / ...); proj_w's ROWS are permuted to match
                    # host-side, so the proj matmul needs no reshuffling
                    at = atp.tile([128, D], F16, name="at")
                    for c in range(8):
                        tpp = tp.tile([128, 128], F16, name="tpp")
                        nc.tensor.transpose(
                            tpp[:], ao4[c // 2][:, (c % 2) * 128:
                                                (c % 2) * 128 + 128],
                            ident[:])
                        nc.vector.tensor_copy(at[:, c * 128:(c + 1) * 128],
                                              tpp[:])
                    ot = outp.tile([128, D], F32, name="ot")
                    for n in range(2):
                        ac = acc.tile([128, STILE], F32, name="ac")
                        for k in range(8):
                            nc.tensor.matmul(
                                ac[:],
                                at[:, k * 128:(k + 1) * 128],
                                pwts[k][:, n * 512:(n + 1) * 512],
                                start=(k == 0), stop=(k == 7))
                        nc.scalar.copy(ot[:, n * 512:(n + 1) * 512], ac[:])
                    if with_projbias:
                        nc.vector.tensor_add(ot[:], ot[:], late["pbb"])
                    nc.sync.dma_start(
                        out=out_ext[s0 + p * 128:s0 + (p + 1) * 128, :],
                        in_=ot[:])

            # ---- main loop over s-tiles (+1 epilogue pass) --------------
            # s-tile N's qkv is emitted before s-tile N-1's attention
            prev_stage = None
            for st in range(n_stiles + 1):
                s0 = st * STILE
                if st == n_stiles:
                    _attention(*prev_stage)
                    break
                xts = xts0 if st == 0 else _load_xt(s0)

                # stage 2a: qT, kT (feature-major, per-m-block tiles; q,k
                # interleaved so attention head-groups unblock early)
                qts, kts = [], []
                for m in range(8):
                    for which, dsts in ((0, qts), (1, kts)):
                        ac = acc.tile([128, STILE], F32, name="ac")
                        col0 = which * D + m * 128
                        for k in range(8):
                            nc.tensor.matmul(
                                ac[:],
                                wts[k][:, col0:col0 + 128],
                                xts[k][:],
                                start=(k == 0), stop=(k == 7))
                        dt = qktp.tile([128, STILE], F16, name="qk",
                                       tag=f"qk{which}{m}")
                        if with_qkbias:
                            nc.scalar.activation(
                                dt[:], ac[:],
                                mybir.ActivationFunctionType.Identity,
                                bias=qkb[:, which * 8 + m:which * 8 + m + 1])
                        else:
                            nc.scalar.copy(dt[:], ac[:])
                        dsts.append(dt)

                if st == 0:
                    _late_consts()

                # stage 2b: v (seq-major, ones column appended per head)
                vts = []
                for b in range(NBLK):
                    vt = vap.tile([128, H * (HD + 1)], F16, name="vt")
                    vt3 = vt.rearrange("p (h c) -> p h c", c=HD + 1)
                    nc.vector.memset(vt3[:, :, HD:HD + 1], 1.0)
                    for n in range(2):
                        ac = acc.tile([128, STILE], F32, name="ac")
                        for k in range(8):
                            nc.tensor.matmul(
                                ac[:],
                                xts[k][:, b * 128:b * 128 + 128],
                                wts[k][:, 2 * D + n * 512:2 * D + (n + 1) * 512],
                                start=(k == 0), stop=(k == 7))
                        nc.vector.tensor_copy(
                            vt3[:, n * 8:(n + 1) * 8, 0:HD],
                            ac.rearrange("p (h c) -> p h c", c=HD))
                    vts.append(vt)

                cur = (vts, qts, kts, s0)
                if prev_stage is not None:
                    _attention(*prev_stage)
                prev_stage = cur

    nc.compile()
    return nc


def _host_prep(x, qkv_w, qkv_b, proj_w, proj_b, rel_bias):
    """Fold scale/biases, cast to fp16, build the exp'd blocked bias table."""
    scale = 1.0 / np.sqrt(HD)
    qkv_w_s = np.asarray(qkv_w, dtype=np.float64).copy()
    qkv_w_s[:, :D] *= scale
    qkv_b = np.asarray(qkv_b, dtype=np.float64)
    qkv_b_s = qkv_b.copy()
    qkv_b_s[:D] *= scale

    # rel-bias expanded to [H, W, W], packed into the transposed,
    # window-pair [128 (k), H*128 (h-major, q)] table, then EXP'd:
    # att = exp(scores) * exp(bias); masked cross-window entries become
    # exactly 0.
    rb = np.asarray(rel_bias, dtype=np.float32)
    coords = np.arange(W)
    rel = coords[:, None] - coords[None, :] + (W - 1)      # [q, k]
    bias_hqk = rb[rel].transpose(2, 0, 1)                  # [H, q, k]
    b2 = np.full((H, 128, 128), -10000.0, dtype=np.float32)  # [H, k2, q2]
    bias_kq = bias_hqk.transpose(0, 2, 1)                  # [H, k, q]
    b2[:, :64, :64] = bias_kq
    b2[:, 64:, 64:] = bias_kq
    expb16 = np.ascontiguousarray(
        np.exp(b2.transpose(1, 0, 2)).reshape(128, H * 128)).astype(np.float16)

    # v-bias commutes through attention (rows sum to 1) -> fold into proj_b
    proj_b_eff = (qkv_b[2 * D:] @ np.asarray(proj_w, dtype=np.float64)
                  + np.asarray(proj_b, dtype=np.float64))

    # attn_out features arrive in parity-group order (see _attention):
    # block c2 holds the head pair PAIRS[c2]; permute proj_w rows to match
    pairs = [(0, 2), (4, 6), (1, 3), (5, 7),
             (8, 10), (12, 14), (9, 11), (13, 15)]
    perm = np.array([h * HD + d for pr in pairs for h in pr
                     for d in range(HD)])
    shared = {
        "qkvw16": qkv_w_s.astype(np.float16),
        "projw16": np.ascontiguousarray(
            np.asarray(proj_w)[perm]).astype(np.float16),
        "expb16": expb16,
    }
    qk_bias = qkv_b_s[:2 * D]
    with_qkbias = bool(np.any(qk_bias))
    if with_qkbias:
        shared["qkb"] = np.ascontiguousarray(
            qk_bias.reshape(16, 128, 1).astype(np.float32))
    with_projbias = bool(np.any(proj_b_eff))
    if with_projbias:
        shared["projb_bcast"] = np.broadcast_to(
            proj_b_eff.astype(np.float32), (128, D)).copy()
    return shared, with_qkbias, with_projbias


_NC_CACHE = {}


def kernel(x, qkv_w, qkv_b, proj_w, proj_b, rel_bias):
    x = np.asarray(x)
    shared, wqk, wpb = _host_prep(x, qkv_w, qkv_b, proj_w, proj_b, rel_bias)

    key = (wqk, wpb)
    if key not in _NC_CACHE:
        _NC_CACHE[key] = _build(NST, wqk, wpb)
    nc = _NC_CACHE[key]

    # feature-major xT per batch element (seq stays the fast axis on chip)
    xt16 = np.ascontiguousarray(
        x.astype(np.float16).transpose(0, 2, 1))          # [B, D, S]
    in_maps = [dict(shared, xt16=xt16[i]) for i in range(B)]
    res = run_bass_kernel_spmd(nc, in_maps, list(range(B)))
    return np.stack([res.results[i]["out"] for i in range(B)], axis=0)


if __name__ == "__main__":
    rng = np.random.default_rng(0)
    x = rng.standard_normal((B, S, D), dtype=np.float32)
    qkv_w = rng.standard_normal((D, 3 * D), dtype=np.float32) / np.sqrt(D)
    proj_w = rng.standard_normal((D, D), dtype=np.float32) / np.sqrt(D)
    out = kernel(x, qkv_w, np.zeros(3 * D, np.float32), proj_w,
                 np.zeros(D, np.float32),
                 rng.standard_normal((2 * W - 1, H), dtype=np.float32) * 0.02)
    print(out.shape, out.dtype)
